# revision 6
# baseline (speedup 1.0000x reference)
"""KAN layer (B=8192, IN_F=OUT_F=1024, GRID=5) on 8 Trainium2 cores.

Math: Y[b,o] = W0[o]*silu(x) + spline_o(clip(x,-1,1)) + b[o], x = X[b,o]
(idx_in = arange(O) % IN_F is the identity here since O == IN_F).

The degree-1 B-spline on the uniform 5-knot grid over [-1,1] is rewritten in
the clipped-ramp (segment) basis: with knots s_j in {-1,-0.5,0,0.5} and
t_j = clip(x, s_j, s_j+0.5),  spline(clip(x)) = A'' + sum_j m_j * t_j,
m_j = 2*(c_{j+1}-c_j).  Each t_j is ONE tensor_scalar(min,max) from the raw
(unclipped) x — no shared clip pass.  Folding W1 and b gives
  Y^T[o,:] = W0*silu(x) + sum_j wm_j*t_j + A'.

Layout: edges on SBUF partitions (X pre-transposed AND cast to fp16 on host),
batch on the free dim, data-parallel over batch across the 8 cores.  Per
128-edge block the weighted sum of the 5 feature maps runs on TensorE as
diagonal-stationary matmuls accumulating in PSUM: (t0,t1) ride ONE fp8e4
DoubleRow matmul (2 k-tiles, 0.5 cyc/col), t2/t3/silu are fp16 (1 cyc/col).
All diagonal stationaries are precomputed on host and DMA'd (zero engine
cost).  ScalarE computes silu; VectorE computes the four t-features (t2/t3
fp16->fp16 at 4x DVE rate, t0/t1 fp16->fp8 at 2x); PSUM evacuation (+ per-
edge bias A', fp32->fp16) splits between ScalarE and GpSimd.  Output is
stored as fp16 and widened on host.  DMA: x on the Sync queue, constants on
the Scalar/Vector queues, stores on GpSimd SWDGE.
"""
import sys

for _p in ("/root/.axon_site", "/root/.axon_site/_ro/trn_rl_repo", "/root/.axon_site/_ro/pypackages"):
    if _p not in sys.path:
        sys.path.append(_p)

import numpy as np
import ml_dtypes

import concourse.bacc as bacc
import concourse.tile as tile
from concourse import mybir
from concourse.bass_utils import run_bass_kernel_spmd

B, IN_F, OUT_F, GRID = 8192, 1024, 1024, 5
N_CORES = 8
B_SHARD = B // N_CORES          # 1024 batch rows per core
EB = OUT_F // 128               # 8 edge blocks
CHUNK = 512                     # one PSUM bank of fp32

DVE_EVAC = (2, 4)               # blocks evacuated by VectorE (rest: ScalarE;
                                # GPSIMD cannot read PSUM on TRN2)
POOL_T1 = (0, 1, 2, 3, 4, 5)    # blocks whose t1 feature GpSimd produces
X_CHUNKS = ((0, 1), (1, 2), (2, 4), (4, 6), (6, 8))   # input DMA block spans

_nc_cache = None


def _build():
    f32 = mybir.dt.float32
    f16 = mybir.dt.float16
    f8 = mybir.dt.float8e4
    AF = mybir.ActivationFunctionType
    OP = mybir.AluOpType
    PM = mybir.MatmulPerfMode
    nc = bacc.Bacc("TRN2", target_bir_lowering=False, debug=False)
    xt = nc.dram_tensor("xt", [OUT_F, B_SHARD], f16, kind="ExternalInput").ap()
    dr8 = nc.dram_tensor("dr8", [128, EB, 2, 128], f8, kind="ExternalInput").ap()
    d16 = nc.dram_tensor("d16", [128, EB, 3, 128], f16, kind="ExternalInput").ap()
    apr = nc.dram_tensor("apr", [128, EB], f32, kind="ExternalInput").ap()
    yt = nc.dram_tensor("yt", [OUT_F, B_SHARD], f16, kind="ExternalOutput").ap()

    xt3 = xt.rearrange("(n p) d -> p n d", p=128)   # [128, EB, B_SHARD]
    yt3 = yt.rearrange("(n p) d -> p n d", p=128)

    with tile.TileContext(nc) as tc:
        with tc.tile_pool(name="const", bufs=1) as const_pool, \
             tc.tile_pool(name="xin", bufs=3) as xin_pool, \
             tc.tile_pool(name="f8p", bufs=3) as f8_pool, \
             tc.tile_pool(name="f16p", bufs=3) as f16_pool, \
             tc.tile_pool(name="silup", bufs=3) as silu_pool, \
             tc.tile_pool(name="yout", bufs=3) as yout_pool, \
             tc.tile_pool(name="ps", bufs=3, space="PSUM") as psum_pool, \
             tc.tile_pool(name="pswarm", bufs=1, space="PSUM") as warm_pool:
            dr8_t = const_pool.tile([128, EB, 2, 128], f8)
            d16_t = const_pool.tile([128, EB, 3, 128], f16)
            apr_t = const_pool.tile([128, EB], f32)
            nc.scalar.dma_start(dr8_t[:], dr8[:, :, :, :])
            nc.scalar.dma_start(apr_t[:], apr[:, :])
            nc.scalar.dma_start(d16_t[:], d16[:, :, :, :])

            # HAM warm-up: dummy matmuls on scratch SBUF so the PE clock
            # gate opens before the first real matmul arrives
            scratch = const_pool.tile([128, CHUNK], f16)
            nc.vector.memset(scratch[:], 0.0)
            ps_warm = warm_pool.tile([128, CHUNK], f32, tag="pswarm", name="pswarm")
            for _ in range(9):
                nc.tensor.matmul(ps_warm[:], scratch[:, 0:128], scratch[:],
                                 start=True, stop=True, skip_group_check=True)

            for span_i, (b0, b1) in enumerate(X_CHUNKS):
                nb = b1 - b0
                x_t = xin_pool.tile([128, nb, B_SHARD], f16, tag=f"x{nb}",
                                    name=f"x_{b0}")
                nc.sync.dma_start(x_t[:], xt3[:, b0:b1, :])
                for h in range(nb):
                    e = b0 + h
                    xv = x_t[:, h, :]
                    # features: t_j = clip(x, s_j, s_j + 0.5)
                    fpair = f8_pool.tile([128, 2, B_SHARD], f8, tag="fp",
                                         name=f"fp_{e}")
                    nc.vector.tensor_scalar(fpair[:, 0, :], xv, -0.5, -1.0,
                                            OP.min, OP.max)
                    t1_eng = nc.gpsimd if e in POOL_T1 else nc.vector
                    t1_eng.tensor_scalar(fpair[:, 1, :], xv, 0.0, -0.5,
                                         OP.min, OP.max)
                    t2 = f16_pool.tile([128, B_SHARD], f16, tag="t2",
                                       name=f"t2_{e}")
                    nc.vector.tensor_scalar(t2[:], xv, 0.5, 0.0, OP.min, OP.max)
                    t3 = f16_pool.tile([128, B_SHARD], f16, tag="t3",
                                       name=f"t3_{e}")
                    nc.vector.tensor_scalar(t3[:], xv, 1.0, 0.5, OP.min, OP.max)
                    silu_t = silu_pool.tile([128, B_SHARD], f16, tag="sl",
                                            name=f"sl_{e}")
                    nc.scalar.activation(silu_t[:], xv, AF.Silu)

                    ps = psum_pool.tile([128, B_SHARD], f32, tag="ps",
                                        name=f"ps_{e}")
                    for t in range(2):
                        cs = slice(t * CHUNK, (t + 1) * CHUNK)
                        nc.tensor.matmul(ps[:, cs], dr8_t[:, e, :, :],
                                         fpair[:, :, cs], start=True, stop=False,
                                         perf_mode=PM.DoubleRow,
                                         skip_group_check=True)
                    for j, ft in enumerate((t2, t3)):
                        for t in range(2):
                            cs = slice(t * CHUNK, (t + 1) * CHUNK)
                            nc.tensor.matmul(ps[:, cs], d16_t[:, e, j, :],
                                             ft[:, cs], start=False, stop=False,
                                             skip_group_check=True)
                    for t in range(2):
                        cs = slice(t * CHUNK, (t + 1) * CHUNK)
                        nc.tensor.matmul(ps[:, cs], d16_t[:, e, 2, :],
                                         silu_t[:, cs], start=False, stop=True,
                                         skip_group_check=True)

                    yo = yout_pool.tile([128, B_SHARD], f16, tag="yo",
                                        name=f"yo_{e}")
                    if e in DVE_EVAC:
                        nc.vector.tensor_scalar_add(yo[:], ps[:],
                                                    apr_t[:, e:e + 1])
                    else:
                        nc.scalar.activation(yo[:], ps[:], AF.Identity,
                                             bias=apr_t[:, e:e + 1], scale=1.0)
                    nc.gpsimd.dma_start(yt3[:, e:e + 1, :], yo[:, None, :])
    nc.compile()
    return nc


def _host_prep(X, coeffs, W, b):
    c = coeffs.astype(np.float64)
    Wd = W.astype(np.float64)
    bd = b.astype(np.float64)
    m = 2.0 * (c[:, 1:] - c[:, :-1])             # [O, 4] slopes per unit x
    w1 = Wd[:, 1]
    wm = w1[:, None] * m                          # [O, 4] per-edge t weights
    s = np.array([-1.0, -0.5, 0.0, 0.5])
    aprime = bd + w1 * c[:, 0] - (wm * s[None, :]).sum(1)

    eye = np.eye(128, dtype=np.float64)
    wmT = wm.reshape(EB, 128, 4).transpose(1, 0, 2)        # [k, e, j]
    w0T = Wd[:, 0].reshape(EB, 128).transpose(1, 0)        # [k, e]
    dr8 = (wmT[:, :, 0:2, None] * eye[:, None, None, :]).astype(
        ml_dtypes.float8_e4m3)                             # [128, EB, 2, 128]
    d16 = np.concatenate([wmT[:, :, 2:4], w0T[:, :, None]], axis=2)
    d16 = (d16[:, :, :, None] * eye[:, None, None, :]).astype(np.float16)
    apr = aprime.reshape(EB, 128).transpose(1, 0).astype(np.float32)
    return dr8, d16, apr


def make_in_maps(X, coeffs, W, b):
    dr8, d16, apr = _host_prep(X, coeffs, W, b)
    x16 = X.astype(np.float16)
    in_maps = []
    for c in range(N_CORES):
        xt_shard = np.ascontiguousarray(x16[c * B_SHARD:(c + 1) * B_SHARD, :].T)
        in_maps.append({"xt": xt_shard, "dr8": dr8, "d16": d16, "apr": apr})
    return in_maps


def kernel(X, coeffs, W, b):
    global _nc_cache
    if _nc_cache is None:
        _nc_cache = _build()
    nc = _nc_cache

    in_maps = make_in_maps(X, coeffs, W, b)
    res = run_bass_kernel_spmd(nc, in_maps, core_ids=list(range(N_CORES)))
    Y = np.empty((B, OUT_F), dtype=np.float32)
    for c in range(N_CORES):
        Y[c * B_SHARD:(c + 1) * B_SHARD, :] = res.results[c]["yt"].T.astype(np.float32)
    return Y
